# revision 13
# baseline (speedup 1.0000x reference)
"""Trainium2 Bass kernel for a binarized (1w1a) ResNet BasicBlock.

Computation (eval mode):
    out = hardtanh(bn2(conv2(sign(out1)) * alpha2) + x)
    out1 = hardtanh(bn1(conv1(sign(x)) * alpha1))
with conv_k a 3x3 stride-1 pad-1 conv whose weights are binarized to
sign(w - rowmean(w)).  Both matmul operands are +-1, which is exact in
fp8e4m3; products are +-1 and PSUM accumulation is fp32, so the conv
arithmetic is exact.

Layout / strategy:
 - Data-parallel over batch N=64 -> 8 images per NeuronCore.
 - Channels 256 = 2 chunks of 128 partitions.  DoubleRow fp8 matmuls
   contract over both chunks at once (K=256): lhsT [128, 2, 128],
   rhs [128, 2, rows, 32] (4D AP: padded rows at stride 34).
 - Each conv = 9 shifted matmuls (3x3 taps) accumulated into PSUM per
   (co_chunk, 16-row half).  Image-boundary taps use reduced row ranges;
   PSUM per-element has_written bits make partial-coverage accumulation
   correct as long as the first matmul covers the full half.
 - conv1 -> bn -> hardtanh -> sign fuses to one ACT op: sign(s1*psum + t1).
 - Final: ACT affine (s2*psum + t2), DVE residual add + fused min/max clamp.
"""

import numpy as np

import concourse.bass as bass
import concourse.mybir as mybir
import concourse.tile as tile
from concourse import bacc
from concourse.bass_utils import run_bass_kernel_spmd
from concourse.masks import make_identity

N_CORES = 8
IMGS = 8  # images per core
CH = 2  # channel chunks of 128
P = 128
H = 32
W = 32
PIX = H * W
WPAD = W + 2  # row stride 34: [pad, w0..w31, pad]
BAL = 2 * WPAD * H  # 1088; padded to 1104 for 16B chunk stride
BASTRIDE = 1104
HALF = 16  # rows per output half (psum free = 16*32 = 512)
EPS = 1e-5
FP = mybir.dt.float32
BF = mybir.dt.bfloat16
F8 = mybir.dt.float8e4
AF = mybir.ActivationFunctionType
DR = mybir.MatmulPerfMode.DoubleRow


def _tap_rows(hs, dh):
    """Valid local output rows [r0, r1) for tap row-offset dh, half start hs."""
    r0 = max(0, -(hs + dh))
    r1 = min(HALF, H - hs - dh)
    return r0, r1


# taps ordered dh=0 first so the first matmul of each accumulation group
# covers the full half (start=True clears the whole bank's has_written bits)
TAPS = sorted(range(9), key=lambda t: abs(t // 3 - 1))


def _emit_conv(nc, pspool, wdr, conv, ba, consumer):
    """One binarized 3x3 conv over one image (DoubleRow fp8).

    ba: [P, 2, BASTRIDE] fp8 tile; chunk i at [:, i, :], rows at stride 34.
    consumer(c, h, hs, ps) reads the [P, 512] fp32 PSUM tile.
    """
    for c in range(CH):
        for h in range(2):
            hs = h * HALF
            ps = pspool.tile([P, HALF * W], FP, tag="ps", name=f"ps{conv}_{c}_{h}")
            for it, t in enumerate(TAPS):
                dh, dw = t // 3 - 1, t % 3 - 1
                r0, r1 = _tap_rows(hs, dh)
                off = (hs + r0 + dh) * WPAD + dw + 1
                rhs = ba[:, :, off : off + (r1 - r0) * WPAD].rearrange(
                    "p i (r w) -> p i r w", r=r1 - r0
                )[:, :, :, 0:W]
                nc.tensor.matmul(
                    ps[:, r0 * W : r1 * W],
                    wdr[(conv, t, c)],
                    rhs,
                    start=(it == 0),
                    stop=(it == 8),
                    perf_mode=DR,
                    skip_group_check=True,
                )
            consumer(c, h, hs, ps)


def build_program(loop_r=None):
    """loop_r: if set, wrap the per-image pipeline in For_i(0, loop_r) —
    benchmarking only (re-processes the same images each iteration)."""
    nc = bacc.Bacc("TRN2", target_bir_lowering=False, debug=False, num_devices=N_CORES)

    x_ext = nc.dram_tensor("x", [IMGS, CH, P, PIX], FP, kind="ExternalInput").ap()
    w_ext = {}
    bn_ext = {}
    for i in (1, 2):
        # transposed layout from host: [ci_chunk, ci%128, co, tap]
        w_ext[i] = nc.dram_tensor(
            f"conv{i}_w", [CH, P, CH * P, 9], FP, kind="ExternalInput"
        ).ap()
        for nm in ("alpha", "gamma", "beta", "mean", "var"):
            bn_ext[(i, nm)] = nc.dram_tensor(
                f"bn{i}_{nm}", [CH, P, 1], FP, kind="ExternalInput"
            ).ap()
    out_ext = nc.dram_tensor("out", [IMGS, CH, P, PIX], FP, kind="ExternalOutput").ap()

    with tile.TileContext(nc) as tc:
        from contextlib import ExitStack

        with ExitStack() as ctx:
            singles = ctx.enter_context(tc.tile_pool(name="singles", bufs=1))
            wpool = ctx.enter_context(tc.tile_pool(name="wpool", bufs=1))
            wstage = ctx.enter_context(tc.tile_pool(name="wstage", bufs=2))
            xpool = ctx.enter_context(tc.tile_pool(name="xpool", bufs=3))
            bapool = ctx.enter_context(tc.tile_pool(name="bapool", bufs=3))
            vpool = ctx.enter_context(tc.tile_pool(name="vpool", bufs=6))
            pspool = ctx.enter_context(tc.tile_pool(name="psum", bufs=8, space="PSUM"))

            eps_t = singles.tile([P, 1], FP)
            nc.vector.memset(eps_t, EPS)
            ones1 = singles.tile([1, P], FP)
            nc.vector.memset(ones1, 1.0)
            ones128 = nc.const_aps.aps[(FP, 1.0)]  # [128, 1] of 1.0

            # ---- weight prep (transposed host layout [ci, co, tap]):
            # bw = sign(w - mean_over_(ci,tap)[co]).  Column sums via two
            # ones-matmuls into PSUM, mean broadcast back over partitions via a
            # rank-1 ones matmul, subtract on DVE (bf16 keeps the sign exact),
            # sign+pack to fp8 DoubleRow lhsT tiles [ci%128, ci//128, co].
            wdr = {}
            for i in (1, 2):
                for c in range(CH):
                    for t in range(9):
                        wdr[(i, t, c)] = wpool.tile(
                            [P, CH, P], F8, tag=f"w{i}_{t}_{c}", name=f"w{i}_{t}_{c}"
                        )
                wTraw = wstage.tile([P, CH, CH * P, 9], FP, tag="wtraw")
                for b in range(CH):
                    nc.sync.dma_start(out=wTraw[:, b], in_=w_ext[i][b])
                tapsum = wstage.tile([P, CH, CH * P], FP, tag="tapsum")
                nc.vector.tensor_reduce(
                    out=tapsum, in_=wTraw, axis=mybir.AxisListType.X,
                    op=mybir.AluOpType.add,
                )
                colsum = pspool.tile([1, CH * P], FP, tag="ps", name="colsum")
                nc.tensor.matmul(colsum, ones128, tapsum[:, 0], start=True, stop=False)
                nc.tensor.matmul(colsum, ones128, tapsum[:, 1], start=False, stop=True)
                negmean = wstage.tile([1, CH * P], FP, tag="negmean")
                nc.scalar.mul(negmean, colsum, -1.0 / (CH * P * 9))
                bc_ps = pspool.tile([P, CH * P], FP, tag="ps", name="bcps")
                nc.tensor.matmul(bc_ps, ones1, negmean, start=True, stop=True)
                diff = wstage.tile([P, CH, CH * P, 9], BF, tag="diff")
                for b in range(CH):
                    nc.vector.tensor_tensor(
                        out=diff[:, b], in0=wTraw[:, b],
                        in1=bc_ps.to_broadcast([P, CH * P, 9]),
                        op=mybir.AluOpType.add,
                    )
                sgn = wstage.tile([P, CH, CH * P, 9], F8, tag="sgn")
                nc.scalar.sign(sgn, diff)
                for c in range(CH):
                    for t in range(9):
                        nc.vector.tensor_copy(
                            out=wdr[(i, t, c)], in_=sgn[:, :, c * P : (c + 1) * P, t]
                        )

            # ---- BN constants: s = alpha*gamma/sqrt(var+eps),
            #                    t = beta - mean*gamma/sqrt(var+eps)
            s_t = {}
            t_t = {}
            for i in (1, 2):
                for c in range(CH):
                    loads = {}
                    for nm in ("alpha", "gamma", "beta", "mean", "var"):
                        tl = singles.tile(
                            [P, 1], FP, tag=f"bn{i}_{nm}_{c}", name=f"bn{i}_{nm}_{c}"
                        )
                        nc.sync.dma_start(out=tl, in_=bn_ext[(i, nm)][c])
                        loads[nm] = tl
                    std = singles.tile([P, 1], FP, tag=f"std{i}_{c}", name=f"std{i}_{c}")
                    nc.scalar.activation(std, loads["var"], AF.Sqrt, bias=eps_t)
                    g = singles.tile([P, 1], FP, tag=f"g{i}_{c}", name=f"g{i}_{c}")
                    nc.vector.reciprocal(g, std)
                    nc.vector.tensor_mul(g, g, loads["gamma"])
                    s = singles.tile([P, 1], FP, tag=f"s{i}_{c}", name=f"s{i}_{c}")
                    nc.vector.tensor_mul(s, g, loads["alpha"])
                    tt = singles.tile([P, 1], FP, tag=f"t{i}_{c}", name=f"t{i}_{c}")
                    nc.vector.tensor_mul(tt, g, loads["mean"])
                    nc.vector.tensor_sub(tt, loads["beta"], tt)
                    s_t[(i, c)] = s
                    t_t[(i, c)] = tt

            # ---- per-image pipeline, software-pipelined emission:
            # conv1(n+1) is emitted before conv2(n) so the PE can fill the
            # ba2-dependency gap of image n with image n+1's conv1 matmuls.
            def emit_front(n):
                """Load x(n), binarize, conv1, produce ba2(n).  Returns state."""
                xt = {}
                for b in range(CH):
                    xt[b] = xpool.tile([P, PIX], FP, tag=f"x{b}", name=f"x{b}")
                    nc.sync.dma_start(out=xt[b], in_=x_ext[n, b])
                ba1 = bapool.tile([P, CH, BASTRIDE], F8, tag="ba1", name="ba1")
                ba2 = bapool.tile([P, CH, BASTRIDE], F8, tag="ba2", name="ba2")
                bav2 = {}
                for b in range(CH):
                    bav1 = ba1[:, b, : H * WPAD].rearrange("p (h w) -> p h w", h=H)
                    bav2[b] = ba2[:, b, : H * WPAD].rearrange("p (h w) -> p h w", h=H)
                    for bav in (bav1, bav2[b]):
                        nc.gpsimd.memset(bav[:, :, 0:1], 0.0)
                        nc.gpsimd.memset(bav[:, :, W + 1 : W + 2], 0.0)
                    nc.scalar.sign(
                        bav1[:, :, 1 : 1 + W],
                        xt[b].rearrange("p (h w) -> p h w", h=H),
                    )

                def conv1_post(c, h, hs, ps):
                    # ba2 = sign(s1 * conv + t1)   (sign(hardtanh(y)) == sign(y))
                    nc.scalar.activation(
                        bav2[c][:, hs : hs + HALF, 1 : 1 + W],
                        ps.rearrange("p (h w) -> p h w", h=HALF),
                        AF.Sign,
                        bias=t_t[(1, c)],
                        scale=s_t[(1, c)],
                    )

                _emit_conv(nc, pspool, wdr, 1, ba1, conv1_post)
                return xt, ba2

            def emit_back(n, state):
                xt, ba2 = state

                def conv2_post(c, h, hs, ps):
                    v = vpool.tile([P, HALF * W], FP, tag="v", name="v")
                    nc.vector.tensor_scalar(
                        out=v, in0=ps, scalar1=s_t[(2, c)], scalar2=t_t[(2, c)],
                        op0=mybir.AluOpType.mult, op1=mybir.AluOpType.add,
                    )
                    nc.vector.tensor_add(v, v, xt[c][:, hs * W : hs * W + HALF * W])
                    nc.vector.tensor_scalar(
                        out=v, in0=v, scalar1=1.0, scalar2=-1.0,
                        op0=mybir.AluOpType.min, op1=mybir.AluOpType.max,
                    )
                    nc.sync.dma_start(
                        out=out_ext[n, c][:, hs * W : hs * W + HALF * W], in_=v
                    )

                _emit_conv(nc, pspool, wdr, 2, ba2, conv2_post)

            def image_pipeline(_iv=None):
                prev = None
                for n in range(IMGS):
                    state = emit_front(n)
                    if prev is not None:
                        emit_back(n - 1, prev)
                    prev = state
                emit_back(IMGS - 1, prev)

            if loop_r is None:
                image_pipeline()
            else:
                with tc.For_i(0, loop_r, 1) as iv:
                    image_pipeline(iv)

    nc.compile()
    return nc


_NC_CACHE = None


def _get_program():
    global _NC_CACHE
    if _NC_CACHE is None:
        _NC_CACHE = build_program()
    return _NC_CACHE


def make_in_maps(inputs):
    x = np.ascontiguousarray(inputs["x"], dtype=np.float32).reshape(
        N_CORES, IMGS, CH, P, PIX
    )
    shared = {}
    for i in (1, 2):
        # [co, ci, kh, kw] -> [ci, co, tap] -> chunked [CH, P, 256, 9]
        shared[f"conv{i}_w"] = np.ascontiguousarray(
            np.asarray(inputs[f"conv{i}_w"], dtype=np.float32)
            .reshape(CH * P, CH * P, 9)
            .transpose(1, 0, 2)
        ).reshape(CH, P, CH * P, 9)
        shared[f"bn{i}_alpha"] = np.ascontiguousarray(
            inputs[f"alpha{i}"], dtype=np.float32
        ).reshape(CH, P, 1)
        for nm in ("gamma", "beta", "mean", "var"):
            shared[f"bn{i}_{nm}"] = np.ascontiguousarray(
                inputs[f"bn{i}_{nm}"], dtype=np.float32
            ).reshape(CH, P, 1)
    return [{"x": x[c], **shared} for c in range(N_CORES)]


def kernel(**inputs):
    nc = _get_program()
    in_maps = make_in_maps(inputs)
    res = run_bass_kernel_spmd(nc, in_maps, list(range(N_CORES)))
    out = np.stack([res.results[c]["out"] for c in range(N_CORES)])
    return out.reshape(N_CORES * IMGS, CH * P, H, W)
